# revision 1
# baseline (speedup 1.0000x reference)
"""GAdapter (GNN message passing + adapter MLP) Bass kernel for Trainium2, 8-core SPMD.

Entry point: kernel(**inputs) -> np.ndarray [1, N, H] float32.

Two-launch design (no collectives).

k1 (per core): LN of own x slab -> eta (residual, f32 out) and
    hM = eta @ (diag(pre_g) M) + pre_b M   (bf16 out)      M = down_w^T up_w^T
Host: concat hM slabs -> full table [N+1, H] bf16 (last row zero sentinel),
    replicate to all cores.
k2 (per core): per 128-row tile, gather hM[dst] for group-packed edge slots,
    build one-hot lhsT per GROUP (G batches share one scalar column: partition k
    is pinned to source row R[k] for the whole group), PE-accumulate into PSUM
    => z2 tile; relu + residual + post-LN; DMA out.
"""

from contextlib import ExitStack
from dataclasses import dataclass

import numpy as np

import concourse.bass as bass
import concourse.tile as tile
from concourse import bacc, mybir

F32 = mybir.dt.float32
BF16 = mybir.dt.bfloat16
I16 = mybir.dt.int16
EPS = 1e-5


@dataclass
class Cfg:
    N: int = 16384
    H: int = 128
    NCORES: int = 8
    G: int = 4        # batches per one-hot group
    NG: int = 9       # groups per 128-row tile (data-dependent max)
    CHUNK: int = 2    # row-tiles per dma_gather call
    use_bM: bool = False
    use_c: bool = False
    use_pre_gb: bool = False
    use_post_gb: bool = False
    oh_bufs: int = 12
    gath_bufs: int = 2

    @property
    def SLAB(self):
        return self.N // self.NCORES

    @property
    def T(self):
        return self.SLAB // 128

    @property
    def NB(self):
        return self.NG * self.G  # batches per tile


def build_k1(cfg: Cfg):
    nc = bacc.Bacc("TRN2", target_bir_lowering=False, debug=False, num_devices=cfg.NCORES)
    H, T = cfg.H, cfg.T
    x_slab = nc.dram_tensor("x_slab", [cfg.SLAB, H], F32, kind="ExternalInput")
    ident_in = nc.dram_tensor("ident", [128, 128], F32, kind="ExternalInput")
    down_w_in = nc.dram_tensor("down_w", [32, H], F32, kind="ExternalInput")
    up_wT_in = nc.dram_tensor("up_wT", [32, H], F32, kind="ExternalInput")
    pre_g_in = nc.dram_tensor("pre_g", [H, 1], F32, kind="ExternalInput")
    pre_b_in = nc.dram_tensor("pre_b", [H, 1], F32, kind="ExternalInput")
    hM_out = nc.dram_tensor("hM", [cfg.SLAB, H], BF16, kind="ExternalOutput")
    eta_out = nc.dram_tensor("eta", [cfg.SLAB, H], F32, kind="ExternalOutput")

    with tile.TileContext(nc) as tc, ExitStack() as ctx:
        const = ctx.enter_context(tc.tile_pool(name="const", bufs=1))
        xin = ctx.enter_context(tc.tile_pool(name="xin", bufs=3))
        stat = ctx.enter_context(tc.tile_pool(name="stat", bufs=4))
        work = ctx.enter_context(tc.tile_pool(name="work", bufs=3))
        psA = ctx.enter_context(tc.tile_pool(name="psA", bufs=3, space="PSUM"))
        psP = ctx.enter_context(tc.tile_pool(name="psP", bufs=2, space="PSUM"))

        ident_t = const.tile([128, 128], F32)
        nc.sync.dma_start(ident_t[:], ident_in[:])
        down_w_t = const.tile([32, H], F32)
        nc.sync.dma_start(down_w_t[:], down_w_in[:])
        up_wT_t = const.tile([32, H], F32)
        nc.sync.dma_start(up_wT_t[:], up_wT_in[:])
        pre_g_t = const.tile([H, 1], F32)
        nc.sync.dma_start(pre_g_t[:], pre_g_in[:])
        eps_t = const.tile([128, 1], F32)
        nc.vector.memset(eps_t[:], EPS)

        psM = psP.tile([128, H], F32, tag="pro")
        nc.tensor.matmul(psM[:], down_w_t[:], up_wT_t[:], start=True, stop=True)
        Mg_t = const.tile([128, H], F32)
        nc.vector.tensor_scalar(Mg_t[:], psM[:], pre_g_t[:], None, mybir.AluOpType.mult)

        if cfg.use_bM or cfg.use_pre_gb:
            ones_1 = const.tile([1, 128], F32)
            nc.vector.memset(ones_1[:], 1.0)
        if cfg.use_bM:
            M_t = const.tile([128, H], F32)
            nc.scalar.activation(M_t[:], psM[:], mybir.ActivationFunctionType.Copy)
            pre_b_t = const.tile([H, 1], F32)
            nc.sync.dma_start(pre_b_t[:], pre_b_in[:])
            ps_bM = psP.tile([1, H], F32, tag="pro")
            nc.tensor.matmul(ps_bM[:], pre_b_t[:], M_t[:], start=True, stop=True)
            bM_t = const.tile([1, H], F32)
            nc.scalar.activation(bM_t[:], ps_bM[:], mybir.ActivationFunctionType.Copy)
        if cfg.use_pre_gb:
            # broadcast tiles of pre_g / pre_b along partitions (for residual h)
            pre_g_r = const.tile([1, H], F32)
            nc.sync.dma_start(pre_g_r[:], pre_g_in.ap().rearrange("h one -> one h"))
            pre_b_r = const.tile([1, H], F32)
            nc.sync.dma_start(pre_b_r[:], pre_b_in.ap().rearrange("h one -> one h"))
            ps_g = psP.tile([128, H], F32, tag="pro")
            nc.tensor.matmul(ps_g[:], ones_1[:], pre_g_r[:], start=True, stop=True)
            gb_t = const.tile([128, H], F32)
            nc.scalar.activation(gb_t[:], ps_g[:], mybir.ActivationFunctionType.Copy)
            ps_b = psP.tile([128, H], F32, tag="pro")
            nc.tensor.matmul(ps_b[:], ones_1[:], pre_b_r[:], start=True, stop=True)
            bb_t = const.tile([128, H], F32)
            nc.scalar.activation(bb_t[:], ps_b[:], mybir.ActivationFunctionType.Copy)

        for t in range(T):
            xt = xin.tile([128, H], F32)
            nc.sync.dma_start(xt[:], x_slab[t * 128 : (t + 1) * 128, :])
            st6 = stat.tile([128, 6], F32, tag="st6")
            nc.vector.bn_stats(st6[:], xt[:])
            mv = stat.tile([128, 2], F32, tag="mv")
            nc.vector.bn_aggr(mv[:], st6[:])
            sd = stat.tile([128, 1], F32, tag="sd")
            nc.scalar.activation(sd[:], mv[:, 1:2], mybir.ActivationFunctionType.Sqrt, bias=eps_t[:])
            rstd = stat.tile([128, 1], F32, tag="rstd")
            nc.vector.reciprocal(rstd[:], sd[:])
            eta = work.tile([128, H], F32, tag="eta")
            nc.vector.tensor_scalar(
                eta[:], xt[:], mv[:, 0:1], rstd[:],
                mybir.AluOpType.subtract, mybir.AluOpType.mult,
            )
            if cfg.use_pre_gb:
                hres = work.tile([128, H], F32, tag="hres")
                nc.vector.tensor_tensor(hres[:], eta[:], gb_t[:], mybir.AluOpType.mult)
                nc.vector.tensor_tensor(hres[:], hres[:], bb_t[:], mybir.AluOpType.add)
                nc.sync.dma_start(eta_out[t * 128 : (t + 1) * 128, :], hres[:])
            else:
                nc.sync.dma_start(eta_out[t * 128 : (t + 1) * 128, :], eta[:])
            psT = psA.tile([128, H], F32, tag="psT")
            nc.tensor.transpose(psT[:], eta[:], ident_t[:])
            etaT = work.tile([128, H], F32, tag="etaT")
            nc.scalar.activation(etaT[:], psT[:], mybir.ActivationFunctionType.Copy)
            pshM = psA.tile([128, H], F32, tag="pshM")
            nc.tensor.matmul(pshM[:], etaT[:], Mg_t[:], start=True, stop=not cfg.use_bM)
            if cfg.use_bM:
                nc.tensor.matmul(pshM[:], ones_1[:], bM_t[:], start=False, stop=True)
            hM_bf = work.tile([128, H], BF16, tag="hMbf")
            nc.scalar.activation(hM_bf[:], pshM[:], mybir.ActivationFunctionType.Copy)
            nc.sync.dma_start(hM_out[t * 128 : (t + 1) * 128, :], hM_bf[:])

    nc.compile()
    return nc


def build_k2(cfg: Cfg):
    nc = bacc.Bacc("TRN2", target_bir_lowering=False, debug=False, num_devices=cfg.NCORES)
    H, T, NG, G, CH = cfg.H, cfg.T, cfg.NG, cfg.G, cfg.CHUNK
    NB = cfg.NB
    assert T % CH == 0
    n_chunks = T // CH
    S_tile = NB * 8  # idx columns per row-tile

    table_in = nc.dram_tensor("table", [cfg.N + 1, H], BF16, kind="ExternalInput")
    eta_in = nc.dram_tensor("eta", [cfg.SLAB, H], F32, kind="ExternalInput")
    idx_plane = nc.dram_tensor("idx_plane", [128, T * S_tile], I16, kind="ExternalInput")
    R_plane = nc.dram_tensor("R_plane", [128, T * NG], F32, kind="ExternalInput")
    iota_in = nc.dram_tensor("iota", [128, 128], BF16, kind="ExternalInput")
    down_b_in = nc.dram_tensor("down_b", [32, 1], F32, kind="ExternalInput")
    up_wT_in = nc.dram_tensor("up_wT", [32, H], F32, kind="ExternalInput")
    up_b_in = nc.dram_tensor("up_b", [1, H], F32, kind="ExternalInput")
    post_g_in = nc.dram_tensor("post_g", [1, H], F32, kind="ExternalInput")
    post_b_in = nc.dram_tensor("post_b", [1, H], F32, kind="ExternalInput")
    y_out = nc.dram_tensor("y", [cfg.SLAB, H], F32, kind="ExternalOutput")

    with tile.TileContext(nc) as tc, ExitStack() as ctx:
        const = ctx.enter_context(tc.tile_pool(name="const", bufs=1))
        etap = ctx.enter_context(tc.tile_pool(name="etap", bufs=3))
        stat = ctx.enter_context(tc.tile_pool(name="stat", bufs=4))
        ohp = ctx.enter_context(tc.tile_pool(name="oh", bufs=cfg.oh_bufs))
        gathp = ctx.enter_context(tc.tile_pool(name="gath", bufs=cfg.gath_bufs))
        outp = ctx.enter_context(tc.tile_pool(name="outp", bufs=3))
        psZ = ctx.enter_context(tc.tile_pool(name="psZ", bufs=4, space="PSUM"))
        psP = ctx.enter_context(tc.tile_pool(name="psP", bufs=2, space="PSUM"))

        iota_t = const.tile([128, 128], BF16)
        nc.sync.dma_start(iota_t[:], iota_in[:])
        idxp_t = const.tile([128, T * S_tile], I16)
        nc.sync.dma_start(idxp_t[:], idx_plane[:])
        Rp_t = const.tile([128, T * NG], F32)
        nc.sync.dma_start(Rp_t[:], R_plane[:])

        if cfg.use_c or cfg.use_post_gb:
            ones_1 = const.tile([1, 128], F32)
            nc.vector.memset(ones_1[:], 1.0)
        if cfg.use_c:
            down_b_t = const.tile([32, 1], F32)
            nc.sync.dma_start(down_b_t[:], down_b_in[:])
            up_wT_t = const.tile([32, H], F32)
            nc.sync.dma_start(up_wT_t[:], up_wT_in[:])
            up_b_t = const.tile([1, H], F32)
            nc.sync.dma_start(up_b_t[:], up_b_in[:])
            ps_c = psP.tile([1, H], F32, tag="pro")
            nc.tensor.matmul(ps_c[:], down_b_t[:], up_wT_t[:], start=True, stop=True)
            c_t = const.tile([1, H], F32)
            nc.vector.tensor_tensor(c_t[:], ps_c[:], up_b_t[:], mybir.AluOpType.add)
        if cfg.use_post_gb:
            post_g_t = const.tile([1, H], F32)
            nc.sync.dma_start(post_g_t[:], post_g_in[:])
            post_b_t = const.tile([1, H], F32)
            nc.sync.dma_start(post_b_t[:], post_b_in[:])
            ps_g2 = psP.tile([128, H], F32, tag="pro")
            nc.tensor.matmul(ps_g2[:], ones_1[:], post_g_t[:], start=True, stop=True)
            postg_b_t = const.tile([128, H], F32)
            nc.scalar.activation(postg_b_t[:], ps_g2[:], mybir.ActivationFunctionType.Copy)
            ps_b2 = psP.tile([128, H], F32, tag="pro")
            nc.tensor.matmul(ps_b2[:], ones_1[:], post_b_t[:], start=True, stop=True)
            postb_b_t = const.tile([128, H], F32)
            nc.scalar.activation(postb_b_t[:], ps_b2[:], mybir.ActivationFunctionType.Copy)

        eps_t = const.tile([128, 1], F32)
        nc.vector.memset(eps_t[:], EPS)

        for cc in range(n_chunks):
            gath = gathp.tile([128, CH * NB, H], BF16)
            s0 = cc * CH * S_tile
            nc.gpsimd.dma_gather(
                out_ap=gath[:],
                in_ap=table_in[:],
                idxs_ap=idxp_t[:, s0 : s0 + CH * S_tile],
                num_idxs=CH * NB * 128,
                num_idxs_reg=CH * NB * 128,
                elem_size=H,
                single_packet=False,
            )
            for ti in range(CH):
                t = cc * CH + ti
                psz = psZ.tile([128, H], F32)
                for g in range(NG):
                    oh = ohp.tile([128, G, 128], BF16, tag="oh")
                    nc.vector.tensor_scalar(
                        oh[:],
                        iota_t.ap().rearrange("p (g n) -> p g n", g=1).broadcast(1, G),
                        Rp_t[:, t * NG + g : t * NG + g + 1],
                        None,
                        mybir.AluOpType.is_equal,
                    )
                    for j in range(G):
                        b = g * G + j
                        nc.tensor.matmul(
                            psz[:],
                            oh[:, j, :],
                            gath[:, ti * NB + b, :],
                            start=(b == 0),
                            stop=(b == NB - 1 and not cfg.use_c),
                        )
                if cfg.use_c:
                    nc.tensor.matmul(psz[:], ones_1[:], c_t[:], start=False, stop=True)
                # epilogue
                eta_t = etap.tile([128, H], F32, tag="eta")
                nc.sync.dma_start(eta_t[:], eta_in[t * 128 : (t + 1) * 128, :])
                v = outp.tile([128, H], F32, tag="v")
                nc.scalar.activation(v[:], psz[:], mybir.ActivationFunctionType.Relu)
                v2 = outp.tile([128, H], F32, tag="v2")
                nc.vector.tensor_tensor(v2[:], v[:], eta_t[:], mybir.AluOpType.add)
                st6b = stat.tile([128, 6], F32, tag="st6b")
                nc.vector.bn_stats(st6b[:], v2[:])
                mvb = stat.tile([128, 2], F32, tag="mvb")
                nc.vector.bn_aggr(mvb[:], st6b[:])
                sdb = stat.tile([128, 1], F32, tag="sdb")
                nc.scalar.activation(sdb[:], mvb[:, 1:2], mybir.ActivationFunctionType.Sqrt, bias=eps_t[:])
                rstdb = stat.tile([128, 1], F32, tag="rstdb")
                nc.vector.reciprocal(rstdb[:], sdb[:])
                yt = outp.tile([128, H], F32, tag="yt")
                nc.vector.tensor_scalar(
                    yt[:], v2[:], mvb[:, 0:1], rstdb[:],
                    mybir.AluOpType.subtract, mybir.AluOpType.mult,
                )
                if cfg.use_post_gb:
                    nc.vector.tensor_tensor(yt[:], yt[:], postg_b_t[:], mybir.AluOpType.mult)
                    nc.vector.tensor_tensor(yt[:], yt[:], postb_b_t[:], mybir.AluOpType.add)
                nc.sync.dma_start(y_out[t * 128 : (t + 1) * 128, :], yt[:])

    nc.compile()
    return nc


# ---------------------------------------------------------------------------
# host-side prep
# ---------------------------------------------------------------------------


def pack_edges(src_s, dst_s, n_tiles, G, N):
    """Group-pack sorted edges. Returns (claim_R [n_tiles, list], claim_dst).

    For each 128-row tile: rows' edges split into claims of <= G edges; claim i
    -> (group i//128, partition i%128). Returns per-tile arrays:
      R[t]   : [n_claims_t]  source row offset (0..127) per claim
      DST[t] : [n_claims_t, G] dst indices (sentinel N where empty)
    """
    Rs, DSTs = [], []
    tile_of = src_s >> 7
    bounds = np.searchsorted(tile_of, np.arange(n_tiles + 1))
    for t in range(n_tiles):
        a, b = bounds[t], bounds[t + 1]
        rr = (src_s[a:b] & 127).astype(np.int64)
        dd = dst_s[a:b]
        # edges sorted by src -> rr sorted; split runs into <=G chunks
        R_list = []
        D_list = []
        start = 0
        n = b - a
        while start < n:
            r = rr[start]
            end = start
            while end < n and rr[end] == r and end - start < G:
                end += 1
            d = np.full(G, N, dtype=np.int64)
            d[: end - start] = dd[start:end]
            R_list.append(r)
            D_list.append(d)
            start = end
        Rs.append(np.array(R_list, dtype=np.int64))
        DSTs.append(np.array(D_list, dtype=np.int64).reshape(-1, G))
    return Rs, DSTs


def prep_inputs(x, edge_index, down_w, down_b, up_w, up_b, pre_g, pre_b, post_g,
                post_b, cfg=None):
    N = x.shape[1]
    H = x.shape[2]
    src = np.asarray(edge_index[0], dtype=np.int64)
    dst = np.asarray(edge_index[1], dtype=np.int64)
    order = np.argsort(src, kind="stable")
    src_s = src[order]
    dst_s = dst[order]
    n_tiles = N // 128

    if cfg is None:
        cfg = Cfg(N=N, H=H)
    G = cfg.G
    Rs, DSTs = pack_edges(src_s, dst_s, n_tiles, G, N)
    NG = max(1, int(np.ceil(max(len(r) for r in Rs) / 128)))
    cfg.NG = NG
    cfg.use_bM = bool(np.any(pre_b != 0))
    cfg.use_c = bool(np.any(down_b != 0) or np.any(up_b != 0))
    cfg.use_pre_gb = bool(np.any(pre_g != 1) or np.any(pre_b != 0))
    cfg.use_post_gb = bool(np.any(post_g != 1) or np.any(post_b != 0))
    T = cfg.T
    NB = cfg.NB

    import ml_dtypes

    iota = np.tile(np.arange(128, dtype=np.float32), (128, 1)).astype(ml_dtypes.bfloat16)
    ident = np.eye(128, dtype=np.float32)
    wT = np.ascontiguousarray(np.asarray(up_w, np.float32).T)

    k1_maps, k2_maps = [], []
    for c in range(cfg.NCORES):
        t0 = c * T
        # per-tile slot arrays: dst_slot [T, NG*128, G], R_slot [T, 128, NG]
        idx_cols = []
        Rp = np.zeros((128, T * NG), np.float32)
        dst_all = np.full((T, NG, G, 128), N, dtype=np.int64)  # [t, g, j, k]
        for tt in range(T):
            R_t = Rs[t0 + tt]
            D_t = DSTs[t0 + tt]  # [n_claims, G]
            nclaims = len(R_t)
            ggrid = np.arange(nclaims) // 128
            kgrid = np.arange(nclaims) % 128
            dst_all[tt, ggrid, :, kgrid] = D_t  # [n_claims, G] -> (g, :, k)
            Rp[kgrid, tt * NG + ggrid] = R_t
        # gather idx order: chunk cc covers tiles [cc*CH, cc*CH+CH);
        # within: tile-major, batch b = g*G+j, partition k: idx[(b*128)+k]
        CH = cfg.CHUNK
        flat_tile = dst_all.transpose(0, 1, 2, 3).reshape(T, NB * 128)  # [t, b*128+k]
        for cc in range(T // CH):
            fl = flat_tile[cc * CH : (cc + 1) * CH].reshape(-1)
            w = fl.reshape(-1, 16).T
            idx_cols.append(np.tile(w, (8, 1)))
        idx_plane = np.concatenate(idx_cols, axis=1).astype(np.int16)

        k1_maps.append({
            "x_slab": np.ascontiguousarray(x[0, c * cfg.SLAB : (c + 1) * cfg.SLAB, :], dtype=np.float32),
            "ident": ident,
            "down_w": np.asarray(down_w, np.float32),
            "up_wT": wT,
            "pre_g": np.asarray(pre_g, np.float32).reshape(H, 1),
            "pre_b": np.asarray(pre_b, np.float32).reshape(H, 1),
        })
        k2_maps.append({
            "idx_plane": np.ascontiguousarray(idx_plane),
            "R_plane": np.ascontiguousarray(Rp),
            "iota": iota,
            "down_b": np.asarray(down_b, np.float32).reshape(-1, 1),
            "up_wT": wT,
            "up_b": np.asarray(up_b, np.float32).reshape(1, H),
            "post_g": np.asarray(post_g, np.float32).reshape(1, H),
            "post_b": np.asarray(post_b, np.float32).reshape(1, H),
        })
    return cfg, k1_maps, k2_maps


def run_full(inputs, cfg=None, runner=None):
    """Complete two-launch execution. runner(nc, in_maps) -> list of out dicts."""
    import ml_dtypes
    from concourse.bass_utils import run_bass_kernel_spmd

    if runner is None:
        def runner(nc, in_maps):
            res = run_bass_kernel_spmd(nc, in_maps, list(range(8)))
            return res.results

    cfg, k1_maps, k2_maps = prep_inputs(**inputs, cfg=cfg)
    nc1 = build_k1(cfg)
    r1 = runner(nc1, k1_maps)
    table = np.concatenate([r1[c]["hM"] for c in range(cfg.NCORES)], axis=0)
    table = np.concatenate([table, np.zeros((1, cfg.H), table.dtype)], axis=0)
    for c in range(cfg.NCORES):
        k2_maps[c]["table"] = table
        k2_maps[c]["eta"] = r1[c]["eta"]
    nc2 = build_k2(cfg)
    r2 = runner(nc2, k2_maps)
    y = np.concatenate([r2[c]["y"] for c in range(cfg.NCORES)], axis=0)
    return y[None]


# ---------------------------------------------------------------------------
# main entry
# ---------------------------------------------------------------------------

_CACHE = {}


def kernel(x, edge_index, down_w, down_b, up_w, up_b, pre_g, pre_b, post_g, post_b):
    import numpy as _np
    from concourse.bass_utils import run_bass_kernel_spmd

    inputs = dict(x=_np.asarray(x), edge_index=_np.asarray(edge_index),
                  down_w=_np.asarray(down_w), down_b=_np.asarray(down_b),
                  up_w=_np.asarray(up_w), up_b=_np.asarray(up_b),
                  pre_g=_np.asarray(pre_g), pre_b=_np.asarray(pre_b),
                  post_g=_np.asarray(post_g), post_b=_np.asarray(post_b))
    cfg, k1_maps, k2_maps = prep_inputs(**inputs)
    key = (cfg.N, cfg.H, cfg.G, cfg.NG, cfg.CHUNK, cfg.use_bM, cfg.use_c,
           cfg.use_pre_gb, cfg.use_post_gb)
    if key not in _CACHE:
        _CACHE[key] = (build_k1(cfg), build_k2(cfg))
    nc1, nc2 = _CACHE[key]
    cores = list(range(cfg.NCORES))
    r1 = run_bass_kernel_spmd(nc1, k1_maps, cores).results
    table = _np.concatenate([r1[c]["hM"] for c in range(cfg.NCORES)], axis=0)
    table = _np.concatenate([table, _np.zeros((1, cfg.H), table.dtype)], axis=0)
    for c in range(cfg.NCORES):
        k2_maps[c]["table"] = table
        k2_maps[c]["eta"] = r1[c]["eta"]
    r2 = run_bass_kernel_spmd(nc2, k2_maps, cores).results
    y = _np.concatenate([r2[c]["y"] for c in range(cfg.NCORES)], axis=0)
    return y[None].astype(_np.float32)
